# revision 11
# baseline (speedup 1.0000x reference)
"""Trainium2 Bass kernel v2 for NNConv-style GNN message passing.

Math (edge_attr == ones):
  h   = relu(x @ lin0_w + lin0_b)                      [N, 32]
  W   = (relu(nn_w1[0] + nn_b1) @ nn_w2 + nn_b2).reshape(32, 32)  (constant!)
  g0  = segment_sum(h[src], dst, N)                    [N, 32]
  out = g0 @ W + h @ conv_root + conv_bias             [N, 32]
  edge_emb = relu((out[src] * out[dst]) @ lin1_w + lin1_b)
  score    = edge_emb @ lin2_w + lin2_b                [E]

v2 mapping to 8 NeuronCores (SPMD, one shared program):
  * edges sorted by dst; core c owns dst in [c*6250, (c+1)*6250)
  * node tables (h, out) are bf16 pair rows [n0 n0 n1 n1] (256B) in local
    DRAM, distributed via XOR-slot remote_dma_broadcast (no collectives):
    slot e on core r holds the shard of core r^e; gather indices are
    precomputed per core with that layout baked in
  * gathers: 4 dst-blocks per dma_gather op (bf16, 256B rows, parity
    select fused across the whole op)
  * scatter-add to dst via one-hot matmuls (one fused iseq per op)
  * cross-core waits live in raw-bass regions between TileContexts
"""
import numpy as np
import ml_dtypes

N_NODES = 50000
N_EDGES = 400000
IN_FEAT = 64
H_DIM = 32
N_CORES = 8
NPC = N_NODES // N_CORES        # 6250 nodes per core
BLOCKS = (NPC + 127) // 128     # 49 dst blocks per core
LAST_BLK_N = NPC - (BLOCKS - 1) * 128   # 106
PAIRS_SLOT = BLOCKS * 64        # 3136 padded pair rows per slot
TAB_ROWS = N_CORES * PAIRS_SLOT  # 25088
P = 128
G_BLK = 3                       # dst blocks per gather op
N_OPS = (BLOCKS + G_BLK - 1) // G_BLK   # 13
CH_BLKS = 24                    # blocks in broadcast chunk 0

BF = ml_dtypes.bfloat16


def _prep(x, edge_index):
    """Host-side sharding/sorting. Returns per-core arrays + structure."""
    src = np.asarray(edge_index[0]).astype(np.int64)
    dst = np.asarray(edge_index[1]).astype(np.int64)
    E = src.size
    order = np.argsort(dst, kind="stable")
    s_s, d_s = src[order], dst[order]
    core = d_s // NPC
    d_loc = d_s - core * NPC
    blk = d_loc // 128
    key = core * BLOCKS + blk
    counts = np.bincount(key, minlength=N_CORES * BLOCKS)
    T_B = max(1, int(np.ceil(counts.max() / 128)))
    T_total = BLOCKS * T_B
    SLOTS = T_total * 128

    starts = np.zeros(N_CORES * BLOCKS + 1, np.int64)
    np.cumsum(counts, out=starts[1:])
    pos_in_blk = np.arange(E) - starts[key]
    slot = blk * (T_B * 128) + pos_in_blk     # slot within the core

    pair = s_s // 2
    owner = pair // (NPC // 2)
    rel = pair - owner * (NPC // 2)

    sp = np.zeros((N_CORES, SLOTS), np.int32)         # table row (sentinel 0)
    bs = np.zeros((N_CORES, SLOTS), np.uint8)         # src parity
    dr = np.full((N_CORES, SLOTS), -1.0, np.float32)  # dst rel in block
    inv = np.full((N_CORES, SLOTS), -1, np.int64)     # original edge id
    for c in range(N_CORES):
        m = core == c
        sl = slot[m]
        sp[c, sl] = (owner[m] ^ c) * PAIRS_SLOT + rel[m]
        bs[c, sl] = (s_s[m] & 1).astype(np.uint8)
        dr[c, sl] = (d_loc[m] - blk[m] * 128).astype(np.float32)
        inv[c, sl] = order[m]

    # gather idx wrap: per op (G_BLK blocks), idx k -> [k%16, k//16],
    # replicated across the 8 Q7 cores (128 partitions)
    gsrc = np.empty((N_CORES, 128, SLOTS // 16), np.int16)
    op_cols = []   # (col0, ncols16, n_idx, nt, nb) per op
    c0 = 0
    for g in range(N_OPS):
        nb = min(G_BLK, BLOCKS - g * G_BLK)
        n_idx = nb * T_B * 128
        op_cols.append((c0, n_idx // 16, n_idx, nb * T_B, nb))
        c0 += n_idx // 16
    for c in range(N_CORES):
        pos = 0
        for (col0, nc16, n_idx, nt, nb) in op_cols:
            a = sp[c, pos:pos + n_idx].reshape(n_idx // 16, 16).T
            gsrc[c, :, col0:col0 + nc16] = np.tile(
                a.astype(np.int16), (8, 1))
            pos += n_idx

    bs_pt = bs.reshape(N_CORES, T_total, 128).transpose(0, 2, 1).copy()
    dr_pt = dr.reshape(N_CORES, T_total, 128).transpose(0, 2, 1)
    dr_pt = dr_pt.astype(BF)
    n3 = (BLOCKS + 2) // 3
    drT = np.zeros((N_CORES, 128, n3 * T_B * 128), np.float32)
    drb = dr.reshape(N_CORES, BLOCKS, T_B * 128)
    for b in range(BLOCKS):
        drT[:, (b % 3) * 32, (b // 3) * T_B * 128:(b // 3 + 1) * T_B * 128] \
            = drb[:, b]
    drT = drT.astype(BF)

    xs = np.asarray(x, np.float32)
    x_sh = np.zeros((N_CORES, BLOCKS * 128, IN_FEAT), np.float32)
    x_sh[:, :NPC] = xs.reshape(N_CORES, NPC, IN_FEAT)
    x_sh = x_sh.astype(BF)

    return dict(T_B=T_B, T_total=T_total, SLOTS=SLOTS, op_cols=op_cols,
                gsrc=gsrc, bs_pt=bs_pt, dr_pt=dr_pt, drT=drT, x_sh=x_sh,
                inv=inv)


def _weights(ins, T_B):
    f32 = np.float32
    g = {k: np.asarray(v) for k, v in ins.items()}
    v = np.maximum(g["nn_w1"][0] + g["nn_b1"], 0.0)
    W = (v @ g["nn_w2"] + g["nn_b2"]).reshape(H_DIM, H_DIM)
    Wcat = np.concatenate([W, g["conv_root"]], 0).astype(BF)      # [64, 32]
    w0 = g["lin0_w"].astype(BF)                                   # [64, 32]
    b0 = np.tile(g["lin0_b"][None, :], (P, 1)).astype(f32)        # [128, 32]
    cb = np.tile(g["conv_bias"][None, :], (P, 1)).astype(f32)     # [128, 32]
    w1 = np.zeros((P, 24), f32)                # block-diag: 3 tiles/chunk
    for k in range(3):
        w1[k * 32:(k + 1) * 32, k * 8:(k + 1) * 8] = g["lin1_w"]
    w1 = w1.astype(BF)
    nt_max = G_BLK * T_B
    b1 = np.tile(np.tile(g["lin1_b"], nt_max)[None, :], (P, 1)).astype(f32)
    w2 = np.tile(np.tile(g["lin2_w"][:, 0], nt_max)[None, :], (P, 1)).astype(f32)
    b2 = np.full((P, 1), float(g["lin2_b"].reshape(-1)[0]), f32)
    iota_f = np.tile(np.arange(P, dtype=f32)[None, :], (P, 1)).astype(BF)
    iota_p = np.ascontiguousarray(
        np.tile(np.arange(P, dtype=f32)[:, None], (1, P))).astype(BF)
    iota_pw = np.ascontiguousarray(np.tile(
        np.arange(P, dtype=f32)[:, None], (1, G_BLK * T_B * 128))).astype(BF)
    on1 = np.ones((P, P), f32).astype(BF)
    return dict(Wcat=Wcat, w0=w0, b0=b0, cb=cb, w1=w1, b1=b1, w2=w2, b2=b2,
                iota_f=iota_f, iota_p=iota_p, iota_pw=iota_pw, on1=on1)


def _build(T_B, k_rep=1, phases=3, p1step=99, p2step=99):
    import concourse.bacc as bacc
    import concourse.mybir as mybir
    import concourse.tile as tile
    from concourse.masks import make_identity

    f32 = mybir.dt.float32
    bf16 = mybir.dt.bfloat16
    T_total = BLOCKS * T_B
    SLOTS = T_total * 128
    EPB = T_B * 128                 # edge slots per block
    nt_max = G_BLK * T_B

    nc = bacc.Bacc("TRN2", target_bir_lowering=False, debug=False,
                   num_devices=N_CORES, num_swdge_queues=4,
                   dynamic_dma_scratch_size=32768)
    dt = nc.dram_tensor
    x_d = dt("x_sh", [BLOCKS * 128, IN_FEAT], bf16, kind="ExternalInput")
    gsrc_d = dt("gsrc", [128, SLOTS // 16], mybir.dt.int16,
                kind="ExternalInput")
    bs_d = dt("bs_pt", [128, T_total], mybir.dt.uint8, kind="ExternalInput")
    dr_d = dt("dr_pt", [128, T_total], bf16, kind="ExternalInput")
    N3 = (BLOCKS + 2) // 3
    drT_d = dt("drT", [128, N3 * EPB], bf16, kind="ExternalInput")
    wc_d = dt("Wcat", [64, 32], bf16, kind="ExternalInput")
    w0_d = dt("w0", [64, 32], bf16, kind="ExternalInput")
    b0_d = dt("b0", [P, 32], f32, kind="ExternalInput")
    cb_d = dt("cb", [P, 32], f32, kind="ExternalInput")
    w1_d = dt("w1", [P, 24], bf16, kind="ExternalInput")
    b1_d = dt("b1", [P, nt_max * 8], f32, kind="ExternalInput")
    w2_d = dt("w2", [P, nt_max * 8], f32, kind="ExternalInput")
    iof_d = dt("iota_f", [P, P], bf16, kind="ExternalInput")
    iop_d = dt("iota_p", [P, P], bf16, kind="ExternalInput")
    iopw_d = dt("iota_pw", [P, nt_max * 128], bf16, kind="ExternalInput")
    on1_d = dt("on1", [P, P], bf16, kind="ExternalInput")
    b2_d = dt("b2", [P, 1], f32, kind="ExternalInput")

    h_full = dt("h_full", [TAB_ROWS * 128], bf16)
    o_full = dt("o_full", [TAB_ROWS * 128], bf16)
    sc_d = dt("scores", [P, T_total], f32, kind="ExternalOutput")

    bypass = mybir.AluOpType.bypass
    add = mybir.AluOpType.add
    mult = mybir.AluOpType.mult
    iseq = mybir.AluOpType.is_equal
    Relu = mybir.ActivationFunctionType.Relu
    Copy = mybir.ActivationFunctionType.Copy
    X = mybir.AxisListType.X

    sem_hr = nc.alloc_semaphore("sem_hr")   # h broadcast arrivals
    sem_or = nc.alloc_semaphore("sem_or")   # o broadcast arrivals
    sem_snd = nc.alloc_semaphore("sem_snd")
    sem_hcp = nc.alloc_semaphore("sem_hcp")
    sem_ocp = nc.alloc_semaphore("sem_ocp")

    # persistent SBUF (raw, visible across TileContexts)
    hs_tab = nc.alloc_sbuf_tensor([P, BLOCKS * 32], bf16)
    ob_tab = nc.alloc_sbuf_tensor([P, BLOCKS * 32], bf16)
    stg_h = nc.alloc_sbuf_tensor([P, 7 * BLOCKS * 32], bf16)
    stg_o = nc.alloc_sbuf_tensor([P, 7 * BLOCKS * 32], bf16)
    hxT = nc.alloc_sbuf_tensor([64, BLOCKS * 128], bf16)
    bs_sb = nc.alloc_sbuf_tensor([P, T_total], mybir.dt.uint8)
    dr_sb = nc.alloc_sbuf_tensor([P, T_total], bf16)
    ident = nc.alloc_sbuf_tensor([P, P], bf16)
    wc_sb = nc.alloc_sbuf_tensor([64, 32], bf16)
    w0_sb = nc.alloc_sbuf_tensor([64, 32], bf16)
    b0_sb = nc.alloc_sbuf_tensor([P, 32], f32)
    cb_sb = nc.alloc_sbuf_tensor([P, 32], f32)
    w1_sb = nc.alloc_sbuf_tensor([P, 24], bf16)
    b1_sb = nc.alloc_sbuf_tensor([P, nt_max * 8], f32)
    w2_sb = nc.alloc_sbuf_tensor([P, nt_max * 8], f32)
    iof_sb = nc.alloc_sbuf_tensor([P, P], bf16)
    iop_sb = nc.alloc_sbuf_tensor([P, P], bf16)
    iopw_sb = nc.alloc_sbuf_tensor([P, nt_max * 128], bf16)
    drT_sb = nc.alloc_sbuf_tensor([P, ((BLOCKS + 2) // 3) * EPB], bf16)
    on1_sb = nc.alloc_sbuf_tensor([P, P], bf16)
    b2_sb = nc.alloc_sbuf_tensor([P, 1], f32)

    def bcast(tab_ap, stg, chunk_cols, ksem):
        """Send tab cols to all 7 peers' stage slots (XOR-slot layout)."""
        c0, c1 = chunk_cols
        for e in range(1, N_CORES):
            pe = e ^ 2 if e & 4 else e      # logical->physical tpb distance
            rdests = [None] * 8
            rdests[pe] = (0, pe)
            nc.gpsimd.remote_dma_broadcast(
                out_ap=stg[:][:, (e - 1) * BLOCKS * 32 + c0:
                              (e - 1) * BLOCKS * 32 + c1],
                in_ap=tab_ap[:][:, c0:c1],
                remote_sem=ksem,
                local_sem=sem_snd,
                rdests=rdests,
            )
        nc.gpsimd.trigger_dma(count=7)

    def own_slot_dma(dram_flat, tab, b0, nb, ksem):
        """Write own blocks b0..b0+nb as [n0 n0 n1 n1] pair rows, slot 0."""
        src = tab[:][:, b0 * 32:(b0 + nb) * 32].rearrange(
            "p (b f) -> p b f", f=32)
        for rep in range(2):
            out = dram_flat[b0 * 8192:(b0 + nb) * 8192].rearrange(
                "(b p f) -> p b f", p=128, f=64)[:, :, rep * 32:rep * 32 + 32]
            nc.sync.dma_start(out=out, in_=src).then_inc(ksem, 16)

    def slot_copy_dma(dram_flat, stg, ksem):
        """Copy the 7 received stage slots into DRAM table slots 1..7."""
        for e in range(1, N_CORES):
            src = stg[:][:, (e - 1) * BLOCKS * 32:e * BLOCKS * 32].rearrange(
                "p (b f) -> p b f", f=32)
            base = e * PAIRS_SLOT * 128
            for rep in range(2):
                out = h_o_view(dram_flat, base)[
                    :, :, rep * 32:rep * 32 + 32]
                nc.sync.dma_start(out=out, in_=src).then_inc(ksem, 16)

    def h_o_view(dram_flat, base):
        return dram_flat[base:base + BLOCKS * 8192].rearrange(
            "(b p f) -> p b f", p=128, f=64)

    with tile.TileContext(nc) as tc0:
        with tc0.tile_pool(name="ld", bufs=1):
            make_identity(nc, ident[:])
            for sb, d in [(bs_sb, bs_d), (dr_sb, dr_d), (wc_sb, wc_d), (w0_sb, w0_d), (b0_sb, b0_d),
                          (cb_sb, cb_d), (w1_sb, w1_d), (b1_sb, b1_d),
                          (w2_sb, w2_d), (iof_sb, iof_d), (iop_sb, iop_d),
                          (iopw_sb, iopw_d), (drT_sb, drT_d),
                          (on1_sb, on1_d), (b2_sb, b2_d)]:
                nc.sync.dma_start(out=sb[:], in_=d[:])

    for rep in range(k_rep):
        # ---------------- phase 0: h = relu(x @ w0 + b0) ---------------
        with tile.TileContext(nc) as tc:
            with (
                tc.tile_pool(name="p0", bufs=3) as p0,
                tc.tile_pool(name="p0p", bufs=2, space="PSUM") as p0p,
            ):
                for b in range(BLOCKS):
                    xt = p0.tile([P, IN_FEAT], bf16, tag="xt")
                    nc.sync.dma_start(
                        out=xt[:], in_=x_d[b * 128:(b + 1) * 128, :])
                    ps_xT = p0p.tile([IN_FEAT, P], bf16, tag="xT")
                    nc.tensor.transpose(out=ps_xT[:], in_=xt[:],
                                        identity=ident[:])
                    xT = p0.tile([IN_FEAT, P], bf16, tag="xTs")
                    nc.scalar.activation(out=xT[:], in_=ps_xT[:], func=Copy)
                    ps_h = p0p.tile([P, 32], f32, tag="h")
                    nc.tensor.matmul(out=ps_h[:], lhsT=xT[:],
                                     rhs=w0_sb[:], start=True, stop=True)
                    hb = p0.tile([P, 32], bf16, tag="hb")
                    nc.vector.tensor_tensor(out=hb[:], in0=ps_h[:],
                                            in1=b0_sb[:], op=add)
                    nc.scalar.activation(
                        out=hs_tab[:][:, b * 32:(b + 1) * 32],
                        in_=hb[:], func=Relu)
                    ps_hT = p0p.tile([32, P], bf16, tag="hT")
                    nc.tensor.transpose(
                        out=ps_hT[:],
                        in_=hs_tab[:][:, b * 32:(b + 1) * 32],
                        identity=ident[:])
                    nc.scalar.activation(
                        out=hxT[:][32:64, b * 128:(b + 1) * 128],
                        in_=ps_hT[:], func=Copy)
                    if b == CH_BLKS - 1:
                        bcast(hs_tab, stg_h, (0, CH_BLKS * 32), sem_hr)
                    if b == BLOCKS - 1:
                        bcast(hs_tab, stg_h,
                              (CH_BLKS * 32, BLOCKS * 32), sem_hr)
        # raw: wait h arrivals, copy stage -> table slots
        own_slot_dma(h_full, hs_tab, 0, BLOCKS, sem_hcp)
        nc.sync.wait_ge(sem_hr, (rep + 1) * 28)
        slot_copy_dma(h_full, stg_h, sem_hcp)
        nc.sync.wait_ge(sem_hcp, (rep + 1) * 16 * 16)

        if phases < 1:
            continue
        # ------- phase 1: gather h[src], aggregate, out table ----------
        with tile.TileContext(nc) as tc:
            with (
                tc.tile_pool(name="p1", bufs=4) as p1,
                tc.tile_pool(name="p1b", bufs=2) as p1b,
                tc.tile_pool(name="p1p", bufs=2, space="PSUM") as p1p,
                tc.tile_pool(name="p1q", bufs=2, space="PSUM") as p1q,
            ):
                for g in range(N_OPS):
                    col0 = g * G_BLK * T_B * 8
                    nb = min(G_BLK, BLOCKS - g * G_BLK)
                    nt = nb * T_B
                    n_idx = nt * 128
                    t0g = g * G_BLK * T_B
                    gi = p1.tile([128, nt_max * 8], mybir.dt.int16, tag="gi")
                    nc.sync.dma_start(
                        out=gi[:, :n_idx // 16],
                        in_=gsrc_d[:, col0:col0 + n_idx // 16])
                    gd = p1.tile([P, nt_max, 128], bf16, tag="gd")
                    for ci, ts in enumerate(range(0, nt, 7)):
                        tn = min(7, nt - ts)
                        nc.gpsimd.dma_gather(
                            gd[:, ts:ts + tn, :], h_full[:].rearrange(
                                "(r c) -> r c", c=128),
                            gi[:, ts * 8:(ts + tn) * 8],
                            tn * 128, tn * 128, 128, queue_num=ci % 4)
                    if p1step < 1:
                        continue
                    mk = bs_sb[:][:, t0g:t0g + nt].unsqueeze(2).to_broadcast(
                        [P, nt, 32])
                    nc.vector.copy_predicated(out=gd[:, :nt, 0:32], mask=mk,
                                              data=gd[:, :nt, 64:96])
                    if p1step < 2:
                        continue
                    oh = p1b.tile([P, nt_max * 128], bf16, tag="oh")
                    nc.vector.tensor_tensor(
                        out=oh[:, :nt * 128].rearrange(
                            "p (t f) -> p t f", f=128),
                        in0=dr_sb[:][:, t0g:t0g + nt].unsqueeze(
                            2).to_broadcast([P, nt, 128]),
                        in1=iof_sb[:].unsqueeze(1).to_broadcast(
                            [P, nt, 128]),
                        op=iseq)
                    if p1step < 3:
                        continue
                    ps_g0T = None
                    for t in range(nt):
                        j = t % T_B
                        b = g * G_BLK + t // T_B
                        if j == 0:
                            ps_g0T = p1p.tile([32, P], f32, tag="g0T")
                        nc.tensor.matmul(
                            out=ps_g0T[:], lhsT=gd[:, t, 0:32],
                            rhs=oh[:, t * 128:(t + 1) * 128],
                            start=(j == 0), stop=(j == T_B - 1))
                        if j == T_B - 1:
                            nc.scalar.activation(
                                out=hxT[:][0:32, b * 128:(b + 1) * 128],
                                in_=ps_g0T[:], func=Copy)
                            ps_o = p1q.tile([P, 32], f32, tag="o")
                            nc.tensor.matmul(
                                out=ps_o[:],
                                lhsT=hxT[:][:, b * 128:(b + 1) * 128],
                                rhs=wc_sb[:], start=True, stop=True)
                            nc.vector.tensor_tensor(
                                out=ob_tab[:][:, b * 32:(b + 1) * 32],
                                in0=ps_o[:], in1=cb_sb[:], op=add)
                    if p1step < 4:
                        continue
                    if g == CH_BLKS // G_BLK - 1:
                        bcast(ob_tab, stg_o, (0, CH_BLKS * 32), sem_or)
                    if g == N_OPS - 1:
                        bcast(ob_tab, stg_o,
                              (CH_BLKS * 32, BLOCKS * 32), sem_or)
        if p1step >= 4:
            own_slot_dma(o_full, ob_tab, 0, BLOCKS, sem_ocp)
            nc.sync.wait_ge(sem_or, (rep + 1) * 28)
            slot_copy_dma(o_full, stg_o, sem_ocp)
            nc.sync.wait_ge(sem_ocp, (rep + 1) * 16 * 16)

        if phases < 2:
            continue
        # ---------------- phase 2: edge scores -------------------------
        with tile.TileContext(nc) as tc:
            with (
                tc.tile_pool(name="p2", bufs=4) as p2,
                tc.tile_pool(name="p2b", bufs=2) as p2b,
                tc.tile_pool(name="p2p", bufs=2, space="PSUM") as p2p,
                tc.tile_pool(name="p2q", bufs=2, space="PSUM") as p2q,
            ):
                for g in range(N_OPS):
                    col0 = g * G_BLK * T_B * 8
                    nb = min(G_BLK, BLOCKS - g * G_BLK)
                    nt = nb * T_B
                    n_idx = nt * 128
                    t0g = g * G_BLK * T_B
                    gi = p2.tile([128, nt_max * 8], mybir.dt.int16, tag="gi2")
                    nc.sync.dma_start(
                        out=gi[:, :n_idx // 16],
                        in_=gsrc_d[:, col0:col0 + n_idx // 16])
                    gd = p2.tile([P, nt_max, 128], bf16, tag="gd2")
                    for ci, ts in enumerate(range(0, nt, 7)):
                        tn = min(7, nt - ts)
                        nc.gpsimd.dma_gather(
                            gd[:, ts:ts + tn, :], o_full[:].rearrange(
                                "(r c) -> r c", c=128),
                            gi[:, ts * 8:(ts + tn) * 8],
                            tn * 128, tn * 128, 128, queue_num=ci % 4)
                    mk = bs_sb[:][:, t0g:t0g + nt].unsqueeze(2).to_broadcast(
                        [P, nt, 32])
                    nc.vector.copy_predicated(out=gd[:, :nt, 0:32], mask=mk,
                                              data=gd[:, :nt, 64:96])
                    if p2step < 2:
                        continue
                    # bc: per block, broadcast dst_rel row to 128 partitions
                    bc = p2b.tile([P, nt_max * 128], bf16, tag="bc")
                    for bi in range(nb):
                        b = g * G_BLK + bi
                        for k in range(3):
                            cw = EPB // 3
                            ps_bc = p2q.tile([P, 512], f32, tag="bc")
                            q = (b % 3) * 32
                            c0d = (b // 3) * EPB
                            nc.tensor.matmul(
                                out=ps_bc[:, :cw],
                                lhsT=on1_sb[q:q + 1, :],
                                rhs=drT_sb[q:q + 1,
                                           c0d + k * cw:c0d + (k + 1) * cw],
                                start=True, stop=True)
                            nc.scalar.activation(
                                out=bc[:, bi * EPB + k * cw:
                                       bi * EPB + (k + 1) * cw],
                                in_=ps_bc[:, :cw], func=Copy)
                    if p2step < 3:
                        continue
                    ohT = p2b.tile([P, nt_max * 128], bf16, tag="ohT")
                    nc.vector.tensor_tensor(
                        out=ohT[:, :nt * 128], in0=bc[:, :nt * 128],
                        in1=iopw_sb[:][:, :nt * 128],
                        op=iseq)
                    if p2step < 4:
                        continue
                    ps_m = p2p.tile([P, nt_max * 8], f32, tag="m")
                    for bi in range(nb):
                        b = g * G_BLK + bi
                        ps_B = p2p.tile([P, T_B, 33], f32, tag="B")
                        for j in range(T_B):
                            nc.tensor.matmul(
                                out=ps_B[:, j, 0:32],
                                lhsT=ohT[:, (bi * T_B + j) * 128:
                                         (bi * T_B + j + 1) * 128],
                                rhs=ob_tab[:][:, b * 32:(b + 1) * 32],
                                start=True, stop=True)
                        if p2step < 5:
                            continue
                        z = p2.tile([P, T_B * 32], bf16, tag="z")
                        nc.vector.tensor_tensor(
                            out=z[:].rearrange("p (t f) -> p t f", f=32),
                            in0=gd[:, bi * T_B:(bi + 1) * T_B, 0:32],
                            in1=ps_B[:, :, 0:32], op=mult)
                        if p2step < 6:
                            continue
                        for k in range(3):
                            ps_zT = p2q.tile([96, P], bf16, tag="zT")
                            nc.tensor.transpose(
                                out=ps_zT[:], in_=z[:, k * 96:(k + 1) * 96],
                                identity=ident[:])
                            zT = p2.tile([96, P], bf16, tag="zTs")
                            nc.scalar.activation(out=zT[:], in_=ps_zT[:],
                                                 func=Copy)
                            if p2step < 7:
                                continue
                            tl0 = bi * T_B + k * 3
                            nc.tensor.matmul(
                                out=ps_m[:, tl0 * 8:(tl0 + 3) * 8],
                                lhsT=zT[:], rhs=w1_sb[0:96, :],
                                start=True, stop=True)
                    if p2step < 8:
                        continue
                    s1 = p2.tile([P, nt_max * 8], f32, tag="s1")
                    nc.vector.tensor_tensor(out=s1[:, :nt * 8],
                                            in0=ps_m[:, :nt * 8],
                                            in1=b1_sb[:][:, :nt * 8], op=add)
                    s1r = p2.tile([P, nt_max * 8], f32, tag="s1r")
                    nc.scalar.activation(out=s1r[:, :nt * 8],
                                         in_=s1[:, :nt * 8], func=Relu)
                    nc.vector.tensor_tensor(out=s1r[:, :nt * 8],
                                            in0=s1r[:, :nt * 8],
                                            in1=w2_sb[:][:, :nt * 8],
                                            op=mult)
                    sc = p2.tile([P, nt_max], f32, tag="sc")
                    nc.vector.reduce_sum(
                        out=sc[:, :nt],
                        in_=s1r[:, :nt * 8].rearrange(
                            "p (t e) -> p t e", e=8),
                        axis=X)
                    sc2 = p2.tile([P, nt_max], f32, tag="sc2")
                    nc.vector.tensor_tensor(
                        out=sc2[:, :nt], in0=sc[:, :nt],
                        in1=b2_sb[:][:, 0:1].to_broadcast([P, nt]), op=add)
                    nc.sync.dma_start(out=sc_d[:, t0g:t0g + nt],
                                      in_=sc2[:, :nt])
    nc.compile()
    return nc


def _in_maps(prep, wts):
    maps = []
    for c in range(N_CORES):
        maps.append({
            "x_sh": prep["x_sh"][c],
            "gsrc": prep["gsrc"][c],
            "bs_pt": prep["bs_pt"][c],
            "dr_pt": prep["dr_pt"][c],
            "drT": prep["drT"][c],
            "Wcat": wts["Wcat"], "w0": wts["w0"], "b0": wts["b0"],
            "cb": wts["cb"], "w1": wts["w1"], "b1": wts["b1"],
            "w2": wts["w2"], "iota_f": wts["iota_f"],
            "iota_p": wts["iota_p"], "iota_pw": wts["iota_pw"],
            "on1": wts["on1"],
            "b2": wts["b2"],
        })
    return maps


def _assemble(results, prep):
    scores = np.empty(N_EDGES, np.float32)
    for c in range(N_CORES):
        flat = np.asarray(results[c]["scores"], np.float32).T.reshape(-1)
        inv = prep["inv"][c]
        m = inv >= 0
        scores[inv[m]] = flat[m]
    return scores


def kernel(**inputs):
    from concourse.bass_utils import run_bass_kernel_spmd
    prep = _prep(inputs["x"], inputs["edge_index"])
    wts = _weights(inputs, prep["T_B"])
    nc = _build(prep["T_B"], k_rep=1)
    res = run_bass_kernel_spmd(nc, _in_maps(prep, wts),
                               list(range(N_CORES)))
    return _assemble(res.results, prep)



# revision 12
# speedup vs baseline: 1.0087x; 1.0087x over previous
"""Trainium2 Bass kernel v2 for NNConv-style GNN message passing.

Math (edge_attr == ones):
  h   = relu(x @ lin0_w + lin0_b)                      [N, 32]
  W   = (relu(nn_w1[0] + nn_b1) @ nn_w2 + nn_b2).reshape(32, 32)  (constant!)
  g0  = segment_sum(h[src], dst, N)                    [N, 32]
  out = g0 @ W + h @ conv_root + conv_bias             [N, 32]
  edge_emb = relu((out[src] * out[dst]) @ lin1_w + lin1_b)
  score    = edge_emb @ lin2_w + lin2_b                [E]

v2 mapping to 8 NeuronCores (SPMD, one shared program):
  * edges sorted by dst; core c owns dst in [c*6250, (c+1)*6250)
  * node tables (h, out) are bf16 pair rows [n0 n0 n1 n1] (256B) in local
    DRAM, distributed via XOR-slot remote_dma_broadcast (no collectives):
    slot e on core r holds the shard of core r^e; gather indices are
    precomputed per core with that layout baked in
  * gathers: 4 dst-blocks per dma_gather op (bf16, 256B rows, parity
    select fused across the whole op)
  * scatter-add to dst via one-hot matmuls (one fused iseq per op)
  * cross-core waits live in raw-bass regions between TileContexts
"""
import numpy as np
import ml_dtypes

N_NODES = 50000
N_EDGES = 400000
IN_FEAT = 64
H_DIM = 32
N_CORES = 8
NPC = N_NODES // N_CORES        # 6250 nodes per core
BLOCKS = (NPC + 127) // 128     # 49 dst blocks per core
LAST_BLK_N = NPC - (BLOCKS - 1) * 128   # 106
PAIRS_SLOT = BLOCKS * 64        # 3136 padded pair rows per slot
TAB_ROWS = N_CORES * PAIRS_SLOT  # 25088
P = 128
G_BLK = 3                       # dst blocks per gather op
N_OPS = (BLOCKS + G_BLK - 1) // G_BLK   # 13
CH_BLKS = 24                    # blocks in broadcast chunk 0

BF = ml_dtypes.bfloat16


def _prep(x, edge_index):
    """Host-side sharding/sorting. Returns per-core arrays + structure."""
    src = np.asarray(edge_index[0]).astype(np.int64)
    dst = np.asarray(edge_index[1]).astype(np.int64)
    E = src.size
    order = np.argsort(dst, kind="stable")
    s_s, d_s = src[order], dst[order]
    core = d_s // NPC
    d_loc = d_s - core * NPC
    blk = d_loc // 128
    key = core * BLOCKS + blk
    counts = np.bincount(key, minlength=N_CORES * BLOCKS)
    T_B = max(1, int(np.ceil(counts.max() / 128)))
    T_total = BLOCKS * T_B
    SLOTS = T_total * 128

    starts = np.zeros(N_CORES * BLOCKS + 1, np.int64)
    np.cumsum(counts, out=starts[1:])
    pos_in_blk = np.arange(E) - starts[key]
    slot = blk * (T_B * 128) + pos_in_blk     # slot within the core

    pair = s_s // 2
    owner = pair // (NPC // 2)
    rel = pair - owner * (NPC // 2)

    sp = np.zeros((N_CORES, SLOTS), np.int32)         # table row (sentinel 0)
    bs = np.zeros((N_CORES, SLOTS), np.uint8)         # src parity
    dr = np.full((N_CORES, SLOTS), -1.0, np.float32)  # dst rel in block
    inv = np.full((N_CORES, SLOTS), -1, np.int64)     # original edge id
    for c in range(N_CORES):
        m = core == c
        sl = slot[m]
        sp[c, sl] = (owner[m] ^ c) * PAIRS_SLOT + rel[m]
        bs[c, sl] = (s_s[m] & 1).astype(np.uint8)
        dr[c, sl] = (d_loc[m] - blk[m] * 128).astype(np.float32)
        inv[c, sl] = order[m]

    # gather idx wrap: per op (G_BLK blocks), idx k -> [k%16, k//16],
    # replicated across the 8 Q7 cores (128 partitions)
    gsrc = np.empty((N_CORES, 128, SLOTS // 16), np.int16)
    op_cols = []   # (col0, ncols16, n_idx, nt, nb) per op
    c0 = 0
    for g in range(N_OPS):
        nb = min(G_BLK, BLOCKS - g * G_BLK)
        n_idx = nb * T_B * 128
        op_cols.append((c0, n_idx // 16, n_idx, nb * T_B, nb))
        c0 += n_idx // 16
    for c in range(N_CORES):
        pos = 0
        for (col0, nc16, n_idx, nt, nb) in op_cols:
            a = sp[c, pos:pos + n_idx].reshape(n_idx // 16, 16).T
            gsrc[c, :, col0:col0 + nc16] = np.tile(
                a.astype(np.int16), (8, 1))
            pos += n_idx

    bs_pt = bs.reshape(N_CORES, T_total, 128).transpose(0, 2, 1).copy()
    dr_pt = dr.reshape(N_CORES, T_total, 128).transpose(0, 2, 1)
    dr_pt = dr_pt.astype(BF)
    n3 = (BLOCKS + 2) // 3
    drT = np.zeros((N_CORES, 128, n3 * T_B * 128), np.float32)
    drb = dr.reshape(N_CORES, BLOCKS, T_B * 128)
    for b in range(BLOCKS):
        drT[:, (b % 3) * 32, (b // 3) * T_B * 128:(b // 3 + 1) * T_B * 128] \
            = drb[:, b]
    drT = drT.astype(BF)

    xs = np.asarray(x, np.float32)
    x_sh = np.zeros((N_CORES, BLOCKS * 128, IN_FEAT), np.float32)
    x_sh[:, :NPC] = xs.reshape(N_CORES, NPC, IN_FEAT)
    x_sh = x_sh.astype(BF)

    return dict(T_B=T_B, T_total=T_total, SLOTS=SLOTS, op_cols=op_cols,
                gsrc=gsrc, bs_pt=bs_pt, dr_pt=dr_pt, drT=drT, x_sh=x_sh,
                inv=inv)


def _weights(ins, T_B):
    f32 = np.float32
    g = {k: np.asarray(v) for k, v in ins.items()}
    v = np.maximum(g["nn_w1"][0] + g["nn_b1"], 0.0)
    W = (v @ g["nn_w2"] + g["nn_b2"]).reshape(H_DIM, H_DIM)
    Wcat = np.concatenate([W, g["conv_root"]], 0).astype(BF)      # [64, 32]
    w0 = g["lin0_w"].astype(BF)                                   # [64, 32]
    b0 = np.tile(g["lin0_b"][None, :], (P, 1)).astype(f32)        # [128, 32]
    cb = np.tile(g["conv_bias"][None, :], (P, 1)).astype(f32)     # [128, 32]
    w1 = np.zeros((P, 24), f32)                # block-diag: 3 tiles/chunk
    for k in range(3):
        w1[k * 32:(k + 1) * 32, k * 8:(k + 1) * 8] = g["lin1_w"]
    w1 = w1.astype(BF)
    nt_max = G_BLK * T_B
    b1 = np.tile(np.tile(g["lin1_b"], nt_max)[None, :], (P, 1)).astype(f32)
    w2 = np.tile(np.tile(g["lin2_w"][:, 0], nt_max)[None, :], (P, 1)).astype(f32)
    b2 = np.full((P, 1), float(g["lin2_b"].reshape(-1)[0]), f32)
    iota_f = np.tile(np.arange(P, dtype=f32)[None, :], (P, 1)).astype(BF)
    iota_p = np.ascontiguousarray(
        np.tile(np.arange(P, dtype=f32)[:, None], (1, P))).astype(BF)
    iota_pw = np.ascontiguousarray(np.tile(
        np.arange(P, dtype=f32)[:, None], (1, G_BLK * T_B * 128))).astype(BF)
    on1 = np.ones((P, P), f32).astype(BF)
    return dict(Wcat=Wcat, w0=w0, b0=b0, cb=cb, w1=w1, b1=b1, w2=w2, b2=b2,
                iota_f=iota_f, iota_p=iota_p, iota_pw=iota_pw, on1=on1)


def _build(T_B, k_rep=1, phases=3, p1step=99, p2step=99):
    import concourse.bacc as bacc
    import concourse.mybir as mybir
    import concourse.tile as tile
    from concourse.masks import make_identity

    f32 = mybir.dt.float32
    bf16 = mybir.dt.bfloat16
    T_total = BLOCKS * T_B
    SLOTS = T_total * 128
    EPB = T_B * 128                 # edge slots per block
    nt_max = G_BLK * T_B

    nc = bacc.Bacc("TRN2", target_bir_lowering=False, debug=False,
                   num_devices=N_CORES, num_swdge_queues=4)
    dt = nc.dram_tensor
    x_d = dt("x_sh", [BLOCKS * 128, IN_FEAT], bf16, kind="ExternalInput")
    gsrc_d = dt("gsrc", [128, SLOTS // 16], mybir.dt.int16,
                kind="ExternalInput")
    bs_d = dt("bs_pt", [128, T_total], mybir.dt.uint8, kind="ExternalInput")
    dr_d = dt("dr_pt", [128, T_total], bf16, kind="ExternalInput")
    N3 = (BLOCKS + 2) // 3
    drT_d = dt("drT", [128, N3 * EPB], bf16, kind="ExternalInput")
    wc_d = dt("Wcat", [64, 32], bf16, kind="ExternalInput")
    w0_d = dt("w0", [64, 32], bf16, kind="ExternalInput")
    b0_d = dt("b0", [P, 32], f32, kind="ExternalInput")
    cb_d = dt("cb", [P, 32], f32, kind="ExternalInput")
    w1_d = dt("w1", [P, 24], bf16, kind="ExternalInput")
    b1_d = dt("b1", [P, nt_max * 8], f32, kind="ExternalInput")
    w2_d = dt("w2", [P, nt_max * 8], f32, kind="ExternalInput")
    iof_d = dt("iota_f", [P, P], bf16, kind="ExternalInput")
    iop_d = dt("iota_p", [P, P], bf16, kind="ExternalInput")
    iopw_d = dt("iota_pw", [P, nt_max * 128], bf16, kind="ExternalInput")
    on1_d = dt("on1", [P, P], bf16, kind="ExternalInput")
    b2_d = dt("b2", [P, 1], f32, kind="ExternalInput")

    h_full = dt("h_full", [TAB_ROWS * 128], bf16)
    o_full = dt("o_full", [TAB_ROWS * 128], bf16)
    sc_d = dt("scores", [P, T_total], f32, kind="ExternalOutput")

    bypass = mybir.AluOpType.bypass
    add = mybir.AluOpType.add
    mult = mybir.AluOpType.mult
    iseq = mybir.AluOpType.is_equal
    Relu = mybir.ActivationFunctionType.Relu
    Copy = mybir.ActivationFunctionType.Copy
    X = mybir.AxisListType.X

    sem_hr = nc.alloc_semaphore("sem_hr")   # h broadcast arrivals
    sem_or = nc.alloc_semaphore("sem_or")   # o broadcast arrivals
    sem_snd = nc.alloc_semaphore("sem_snd")
    sem_hcp = nc.alloc_semaphore("sem_hcp")
    sem_ocp = nc.alloc_semaphore("sem_ocp")

    # persistent SBUF (raw, visible across TileContexts)
    hs_tab = nc.alloc_sbuf_tensor([P, BLOCKS * 32], bf16)
    ob_tab = nc.alloc_sbuf_tensor([P, BLOCKS * 32], bf16)
    stg_h = nc.alloc_sbuf_tensor([P, 7 * BLOCKS * 32], bf16)
    stg_o = nc.alloc_sbuf_tensor([P, 7 * BLOCKS * 32], bf16)
    hxT = nc.alloc_sbuf_tensor([64, BLOCKS * 128], bf16)
    bs_sb = nc.alloc_sbuf_tensor([P, T_total], mybir.dt.uint8)
    dr_sb = nc.alloc_sbuf_tensor([P, T_total], bf16)
    ident = nc.alloc_sbuf_tensor([P, P], bf16)
    wc_sb = nc.alloc_sbuf_tensor([64, 32], bf16)
    w0_sb = nc.alloc_sbuf_tensor([64, 32], bf16)
    b0_sb = nc.alloc_sbuf_tensor([P, 32], f32)
    cb_sb = nc.alloc_sbuf_tensor([P, 32], f32)
    w1_sb = nc.alloc_sbuf_tensor([P, 24], bf16)
    b1_sb = nc.alloc_sbuf_tensor([P, nt_max * 8], f32)
    w2_sb = nc.alloc_sbuf_tensor([P, nt_max * 8], f32)
    iof_sb = nc.alloc_sbuf_tensor([P, P], bf16)
    iop_sb = nc.alloc_sbuf_tensor([P, P], bf16)
    iopw_sb = nc.alloc_sbuf_tensor([P, nt_max * 128], bf16)
    drT_sb = nc.alloc_sbuf_tensor([P, ((BLOCKS + 2) // 3) * EPB], bf16)
    on1_sb = nc.alloc_sbuf_tensor([P, P], bf16)
    b2_sb = nc.alloc_sbuf_tensor([P, 1], f32)

    def bcast(tab_ap, stg, chunk_cols, ksem):
        """Send tab cols to all 7 peers' stage slots (XOR-slot layout)."""
        c0, c1 = chunk_cols
        for e in range(1, N_CORES):
            pe = e ^ 2 if e & 4 else e      # logical->physical tpb distance
            rdests = [None] * 8
            rdests[pe] = (0, pe)
            nc.gpsimd.remote_dma_broadcast(
                out_ap=stg[:][:, (e - 1) * BLOCKS * 32 + c0:
                              (e - 1) * BLOCKS * 32 + c1],
                in_ap=tab_ap[:][:, c0:c1],
                remote_sem=ksem,
                local_sem=sem_snd,
                rdests=rdests,
            )
        nc.gpsimd.trigger_dma(count=7)

    def own_slot_dma(dram_flat, tab, b0, nb, ksem):
        """Write own blocks b0..b0+nb as [n0 n0 n1 n1] pair rows, slot 0."""
        src = tab[:][:, b0 * 32:(b0 + nb) * 32].rearrange(
            "p (b f) -> p b f", f=32)
        for rep in range(2):
            out = dram_flat[b0 * 8192:(b0 + nb) * 8192].rearrange(
                "(b p f) -> p b f", p=128, f=64)[:, :, rep * 32:rep * 32 + 32]
            nc.sync.dma_start(out=out, in_=src).then_inc(ksem, 16)

    def slot_copy_dma(dram_flat, stg, ksem):
        """Copy the 7 received stage slots into DRAM table slots 1..7."""
        for e in range(1, N_CORES):
            src = stg[:][:, (e - 1) * BLOCKS * 32:e * BLOCKS * 32].rearrange(
                "p (b f) -> p b f", f=32)
            base = e * PAIRS_SLOT * 128
            for rep in range(2):
                out = h_o_view(dram_flat, base)[
                    :, :, rep * 32:rep * 32 + 32]
                nc.sync.dma_start(out=out, in_=src).then_inc(ksem, 16)

    def h_o_view(dram_flat, base):
        return dram_flat[base:base + BLOCKS * 8192].rearrange(
            "(b p f) -> p b f", p=128, f=64)

    with tile.TileContext(nc) as tc0:
        with tc0.tile_pool(name="ld", bufs=1):
            make_identity(nc, ident[:])
            for sb, d in [(bs_sb, bs_d), (dr_sb, dr_d), (wc_sb, wc_d), (w0_sb, w0_d), (b0_sb, b0_d),
                          (cb_sb, cb_d), (w1_sb, w1_d), (b1_sb, b1_d),
                          (w2_sb, w2_d), (iof_sb, iof_d), (iop_sb, iop_d),
                          (iopw_sb, iopw_d), (drT_sb, drT_d),
                          (on1_sb, on1_d), (b2_sb, b2_d)]:
                nc.sync.dma_start(out=sb[:], in_=d[:])

    for rep in range(k_rep):
        # ---------------- phase 0: h = relu(x @ w0 + b0) ---------------
        with tile.TileContext(nc) as tc:
            with (
                tc.tile_pool(name="p0", bufs=3) as p0,
                tc.tile_pool(name="p0p", bufs=2, space="PSUM") as p0p,
            ):
                for b in range(BLOCKS):
                    xt = p0.tile([P, IN_FEAT], bf16, tag="xt")
                    nc.sync.dma_start(
                        out=xt[:], in_=x_d[b * 128:(b + 1) * 128, :])
                    ps_xT = p0p.tile([IN_FEAT, P], bf16, tag="xT")
                    nc.tensor.transpose(out=ps_xT[:], in_=xt[:],
                                        identity=ident[:])
                    xT = p0.tile([IN_FEAT, P], bf16, tag="xTs")
                    nc.scalar.activation(out=xT[:], in_=ps_xT[:], func=Copy)
                    ps_h = p0p.tile([P, 32], f32, tag="h")
                    nc.tensor.matmul(out=ps_h[:], lhsT=xT[:],
                                     rhs=w0_sb[:], start=True, stop=True)
                    hb = p0.tile([P, 32], bf16, tag="hb")
                    nc.vector.tensor_tensor(out=hb[:], in0=ps_h[:],
                                            in1=b0_sb[:], op=add)
                    nc.scalar.activation(
                        out=hs_tab[:][:, b * 32:(b + 1) * 32],
                        in_=hb[:], func=Relu)
                    ps_hT = p0p.tile([32, P], bf16, tag="hT")
                    nc.tensor.transpose(
                        out=ps_hT[:],
                        in_=hs_tab[:][:, b * 32:(b + 1) * 32],
                        identity=ident[:])
                    nc.scalar.activation(
                        out=hxT[:][32:64, b * 128:(b + 1) * 128],
                        in_=ps_hT[:], func=Copy)
                    if b == CH_BLKS - 1:
                        bcast(hs_tab, stg_h, (0, CH_BLKS * 32), sem_hr)
                    if b == BLOCKS - 1:
                        bcast(hs_tab, stg_h,
                              (CH_BLKS * 32, BLOCKS * 32), sem_hr)
        # raw: wait h arrivals, copy stage -> table slots
        own_slot_dma(h_full, hs_tab, 0, BLOCKS, sem_hcp)
        nc.sync.wait_ge(sem_hr, (rep + 1) * 28)
        slot_copy_dma(h_full, stg_h, sem_hcp)
        nc.sync.wait_ge(sem_hcp, (rep + 1) * 16 * 16)

        if phases < 1:
            continue
        # ------- phase 1: gather h[src], aggregate, out table ----------
        with tile.TileContext(nc) as tc:
            with (
                tc.tile_pool(name="p1", bufs=4) as p1,
                tc.tile_pool(name="p1b", bufs=2) as p1b,
                tc.tile_pool(name="p1p", bufs=2, space="PSUM") as p1p,
                tc.tile_pool(name="p1q", bufs=2, space="PSUM") as p1q,
            ):
                for g in range(N_OPS):
                    col0 = g * G_BLK * T_B * 8
                    nb = min(G_BLK, BLOCKS - g * G_BLK)
                    nt = nb * T_B
                    n_idx = nt * 128
                    t0g = g * G_BLK * T_B
                    gi = p1.tile([128, nt_max * 8], mybir.dt.int16, tag="gi")
                    nc.sync.dma_start(
                        out=gi[:, :n_idx // 16],
                        in_=gsrc_d[:, col0:col0 + n_idx // 16])
                    gd = p1.tile([P, nt_max, 128], bf16, tag="gd")
                    for ci, ts in enumerate(range(0, nt, 7)):
                        tn = min(7, nt - ts)
                        nc.gpsimd.dma_gather(
                            gd[:, ts:ts + tn, :], h_full[:].rearrange(
                                "(r c) -> r c", c=128),
                            gi[:, ts * 8:(ts + tn) * 8],
                            tn * 128, tn * 128, 128, queue_num=ci % 4)
                    if p1step < 1:
                        continue
                    mk = bs_sb[:][:, t0g:t0g + nt].unsqueeze(2).to_broadcast(
                        [P, nt, 32])
                    nc.vector.copy_predicated(out=gd[:, :nt, 0:32], mask=mk,
                                              data=gd[:, :nt, 64:96])
                    if p1step < 2:
                        continue
                    oh = p1b.tile([P, nt_max * 128], bf16, tag="oh")
                    nc.vector.tensor_tensor(
                        out=oh[:, :nt * 128].rearrange(
                            "p (t f) -> p t f", f=128),
                        in0=dr_sb[:][:, t0g:t0g + nt].unsqueeze(
                            2).to_broadcast([P, nt, 128]),
                        in1=iof_sb[:].unsqueeze(1).to_broadcast(
                            [P, nt, 128]),
                        op=iseq)
                    if p1step < 3:
                        continue
                    ps_g0T = None
                    for t in range(nt):
                        j = t % T_B
                        b = g * G_BLK + t // T_B
                        if j == 0:
                            ps_g0T = p1p.tile([32, P], f32, tag="g0T")
                        nc.tensor.matmul(
                            out=ps_g0T[:], lhsT=gd[:, t, 0:32],
                            rhs=oh[:, t * 128:(t + 1) * 128],
                            start=(j == 0), stop=(j == T_B - 1))
                        if j == T_B - 1:
                            nc.scalar.activation(
                                out=hxT[:][0:32, b * 128:(b + 1) * 128],
                                in_=ps_g0T[:], func=Copy)
                            ps_o = p1q.tile([P, 32], f32, tag="o")
                            nc.tensor.matmul(
                                out=ps_o[:],
                                lhsT=hxT[:][:, b * 128:(b + 1) * 128],
                                rhs=wc_sb[:], start=True, stop=True)
                            nc.vector.tensor_tensor(
                                out=ob_tab[:][:, b * 32:(b + 1) * 32],
                                in0=ps_o[:], in1=cb_sb[:], op=add)
                    if p1step < 4:
                        continue
                    if g == CH_BLKS // G_BLK - 1:
                        bcast(ob_tab, stg_o, (0, CH_BLKS * 32), sem_or)
                    if g == N_OPS - 1:
                        bcast(ob_tab, stg_o,
                              (CH_BLKS * 32, BLOCKS * 32), sem_or)
        if p1step >= 4:
            own_slot_dma(o_full, ob_tab, 0, BLOCKS, sem_ocp)
            nc.sync.wait_ge(sem_or, (rep + 1) * 28)
            slot_copy_dma(o_full, stg_o, sem_ocp)
            nc.sync.wait_ge(sem_ocp, (rep + 1) * 16 * 16)

        if phases < 2:
            continue
        # ---------------- phase 2: edge scores -------------------------
        with tile.TileContext(nc) as tc:
            with (
                tc.tile_pool(name="p2", bufs=4) as p2,
                tc.tile_pool(name="p2b", bufs=2) as p2b,
                tc.tile_pool(name="p2p", bufs=2, space="PSUM") as p2p,
                tc.tile_pool(name="p2q", bufs=2, space="PSUM") as p2q,
            ):
                for g in range(N_OPS):
                    col0 = g * G_BLK * T_B * 8
                    nb = min(G_BLK, BLOCKS - g * G_BLK)
                    nt = nb * T_B
                    n_idx = nt * 128
                    t0g = g * G_BLK * T_B
                    gi = p2.tile([128, nt_max * 8], mybir.dt.int16, tag="gi2")
                    nc.sync.dma_start(
                        out=gi[:, :n_idx // 16],
                        in_=gsrc_d[:, col0:col0 + n_idx // 16])
                    gd = p2.tile([P, nt_max, 128], bf16, tag="gd2")
                    for ci, ts in enumerate(range(0, nt, 7)):
                        tn = min(7, nt - ts)
                        nc.gpsimd.dma_gather(
                            gd[:, ts:ts + tn, :], o_full[:].rearrange(
                                "(r c) -> r c", c=128),
                            gi[:, ts * 8:(ts + tn) * 8],
                            tn * 128, tn * 128, 128, queue_num=ci % 4)
                    mk = bs_sb[:][:, t0g:t0g + nt].unsqueeze(2).to_broadcast(
                        [P, nt, 32])
                    nc.vector.copy_predicated(out=gd[:, :nt, 0:32], mask=mk,
                                              data=gd[:, :nt, 64:96])
                    if p2step < 2:
                        continue
                    # bc: per block, broadcast dst_rel row to 128 partitions
                    bc = p2b.tile([P, nt_max * 128], bf16, tag="bc")
                    for bi in range(nb):
                        b = g * G_BLK + bi
                        for k in range(3):
                            cw = EPB // 3
                            ps_bc = p2q.tile([P, 512], f32, tag="bc")
                            q = (b % 3) * 32
                            c0d = (b // 3) * EPB
                            nc.tensor.matmul(
                                out=ps_bc[:, :cw],
                                lhsT=on1_sb[q:q + 1, :],
                                rhs=drT_sb[q:q + 1,
                                           c0d + k * cw:c0d + (k + 1) * cw],
                                start=True, stop=True)
                            nc.scalar.activation(
                                out=bc[:, bi * EPB + k * cw:
                                       bi * EPB + (k + 1) * cw],
                                in_=ps_bc[:, :cw], func=Copy)
                    if p2step < 3:
                        continue
                    ohT = p2b.tile([P, nt_max * 128], bf16, tag="ohT")
                    nc.vector.tensor_tensor(
                        out=ohT[:, :nt * 128], in0=bc[:, :nt * 128],
                        in1=iopw_sb[:][:, :nt * 128],
                        op=iseq)
                    if p2step < 4:
                        continue
                    ps_m = p2p.tile([P, nt_max * 8], f32, tag="m")
                    for bi in range(nb):
                        b = g * G_BLK + bi
                        ps_B = p2p.tile([P, T_B, 33], f32, tag="B")
                        for j in range(T_B):
                            nc.tensor.matmul(
                                out=ps_B[:, j, 0:32],
                                lhsT=ohT[:, (bi * T_B + j) * 128:
                                         (bi * T_B + j + 1) * 128],
                                rhs=ob_tab[:][:, b * 32:(b + 1) * 32],
                                start=True, stop=True)
                        if p2step < 5:
                            continue
                        z = p2.tile([P, T_B * 32], bf16, tag="z")
                        nc.vector.tensor_tensor(
                            out=z[:].rearrange("p (t f) -> p t f", f=32),
                            in0=gd[:, bi * T_B:(bi + 1) * T_B, 0:32],
                            in1=ps_B[:, :, 0:32], op=mult)
                        if p2step < 6:
                            continue
                        for k in range(3):
                            ps_zT = p2q.tile([96, P], bf16, tag="zT")
                            nc.tensor.transpose(
                                out=ps_zT[:], in_=z[:, k * 96:(k + 1) * 96],
                                identity=ident[:])
                            zT = p2.tile([96, P], bf16, tag="zTs")
                            nc.scalar.activation(out=zT[:], in_=ps_zT[:],
                                                 func=Copy)
                            if p2step < 7:
                                continue
                            tl0 = bi * T_B + k * 3
                            nc.tensor.matmul(
                                out=ps_m[:, tl0 * 8:(tl0 + 3) * 8],
                                lhsT=zT[:], rhs=w1_sb[0:96, :],
                                start=True, stop=True)
                    if p2step < 8:
                        continue
                    s1 = p2.tile([P, nt_max * 8], f32, tag="s1")
                    nc.vector.tensor_tensor(out=s1[:, :nt * 8],
                                            in0=ps_m[:, :nt * 8],
                                            in1=b1_sb[:][:, :nt * 8], op=add)
                    s1r = p2.tile([P, nt_max * 8], f32, tag="s1r")
                    nc.scalar.activation(out=s1r[:, :nt * 8],
                                         in_=s1[:, :nt * 8], func=Relu)
                    nc.vector.tensor_tensor(out=s1r[:, :nt * 8],
                                            in0=s1r[:, :nt * 8],
                                            in1=w2_sb[:][:, :nt * 8],
                                            op=mult)
                    sc = p2.tile([P, nt_max], f32, tag="sc")
                    nc.vector.reduce_sum(
                        out=sc[:, :nt],
                        in_=s1r[:, :nt * 8].rearrange(
                            "p (t e) -> p t e", e=8),
                        axis=X)
                    sc2 = p2.tile([P, nt_max], f32, tag="sc2")
                    nc.vector.tensor_tensor(
                        out=sc2[:, :nt], in0=sc[:, :nt],
                        in1=b2_sb[:][:, 0:1].to_broadcast([P, nt]), op=add)
                    nc.sync.dma_start(out=sc_d[:, t0g:t0g + nt],
                                      in_=sc2[:, :nt])
    nc.compile()
    return nc


def _in_maps(prep, wts):
    maps = []
    for c in range(N_CORES):
        maps.append({
            "x_sh": prep["x_sh"][c],
            "gsrc": prep["gsrc"][c],
            "bs_pt": prep["bs_pt"][c],
            "dr_pt": prep["dr_pt"][c],
            "drT": prep["drT"][c],
            "Wcat": wts["Wcat"], "w0": wts["w0"], "b0": wts["b0"],
            "cb": wts["cb"], "w1": wts["w1"], "b1": wts["b1"],
            "w2": wts["w2"], "iota_f": wts["iota_f"],
            "iota_p": wts["iota_p"], "iota_pw": wts["iota_pw"],
            "on1": wts["on1"],
            "b2": wts["b2"],
        })
    return maps


def _assemble(results, prep):
    scores = np.empty(N_EDGES, np.float32)
    for c in range(N_CORES):
        flat = np.asarray(results[c]["scores"], np.float32).T.reshape(-1)
        inv = prep["inv"][c]
        m = inv >= 0
        scores[inv[m]] = flat[m]
    return scores


def kernel(**inputs):
    from concourse.bass_utils import run_bass_kernel_spmd
    prep = _prep(inputs["x"], inputs["edge_index"])
    wts = _weights(inputs, prep["T_B"])
    nc = _build(prep["T_B"], k_rep=1)
    res = run_bass_kernel_spmd(nc, _in_maps(prep, wts),
                               list(range(N_CORES)))
    return _assemble(res.results, prep)



# revision 13
# speedup vs baseline: 1.7848x; 1.7695x over previous
"""Trainium2 Bass kernel v2 for NNConv-style GNN message passing.

Math (edge_attr == ones):
  h   = relu(x @ lin0_w + lin0_b)                      [N, 32]
  W   = (relu(nn_w1[0] + nn_b1) @ nn_w2 + nn_b2).reshape(32, 32)  (constant!)
  g0  = segment_sum(h[src], dst, N)                    [N, 32]
  out = g0 @ W + h @ conv_root + conv_bias             [N, 32]
  edge_emb = relu((out[src] * out[dst]) @ lin1_w + lin1_b)
  score    = edge_emb @ lin2_w + lin2_b                [E]

v2 mapping to 8 NeuronCores (SPMD, one shared program):
  * edges sorted by dst; core c owns dst in [c*6250, (c+1)*6250)
  * node tables (h, out) are bf16 pair rows [n0 n0 n1 n1] (256B) in local
    DRAM, distributed via XOR-slot remote_dma_broadcast (no collectives):
    slot e on core r holds the shard of core r^e; gather indices are
    precomputed per core with that layout baked in
  * gathers: 4 dst-blocks per dma_gather op (bf16, 256B rows, parity
    select fused across the whole op)
  * scatter-add to dst via one-hot matmuls (one fused iseq per op)
  * cross-core waits live in raw-bass regions between TileContexts
"""
import numpy as np
import ml_dtypes

N_NODES = 50000
N_EDGES = 400000
IN_FEAT = 64
H_DIM = 32
N_CORES = 8
NPC = N_NODES // N_CORES        # 6250 nodes per core
BLOCKS = (NPC + 127) // 128     # 49 dst blocks per core
LAST_BLK_N = NPC - (BLOCKS - 1) * 128   # 106
PAIRS_SLOT = BLOCKS * 64        # 3136 padded pair rows per slot
TAB_ROWS = N_CORES * PAIRS_SLOT  # 25088
P = 128
G_BLK = 3                       # dst blocks per gather op
N_OPS = (BLOCKS + G_BLK - 1) // G_BLK   # 13
CH_BLKS = 24                    # blocks in broadcast chunk 0

BF = ml_dtypes.bfloat16


def _prep(x, edge_index):
    """Host-side sharding/sorting. Returns per-core arrays + structure."""
    src = np.asarray(edge_index[0]).astype(np.int64)
    dst = np.asarray(edge_index[1]).astype(np.int64)
    E = src.size
    order = np.argsort(dst, kind="stable")
    s_s, d_s = src[order], dst[order]
    core = d_s // NPC
    d_loc = d_s - core * NPC
    blk = d_loc // 128
    key = core * BLOCKS + blk
    counts = np.bincount(key, minlength=N_CORES * BLOCKS)
    T_B = max(1, int(np.ceil(counts.max() / 128)))
    T_total = BLOCKS * T_B
    SLOTS = T_total * 128

    starts = np.zeros(N_CORES * BLOCKS + 1, np.int64)
    np.cumsum(counts, out=starts[1:])
    pos_in_blk = np.arange(E) - starts[key]
    slot = blk * (T_B * 128) + pos_in_blk     # slot within the core

    pair = s_s // 2
    owner = pair // (NPC // 2)
    rel = pair - owner * (NPC // 2)

    sp = np.zeros((N_CORES, SLOTS), np.int32)         # table row (sentinel 0)
    bs = np.zeros((N_CORES, SLOTS), np.uint8)         # src parity
    dr = np.full((N_CORES, SLOTS), -1.0, np.float32)  # dst rel in block
    inv = np.full((N_CORES, SLOTS), -1, np.int64)     # original edge id
    for c in range(N_CORES):
        m = core == c
        sl = slot[m]
        sp[c, sl] = (owner[m] ^ c) * PAIRS_SLOT + rel[m]
        bs[c, sl] = (s_s[m] & 1).astype(np.uint8)
        dr[c, sl] = (d_loc[m] - blk[m] * 128).astype(np.float32)
        inv[c, sl] = order[m]

    # gather idx wrap: per op (G_BLK blocks), idx k -> [k%16, k//16],
    # replicated across the 8 Q7 cores (128 partitions)
    gsrc = np.empty((N_CORES, 128, SLOTS // 16), np.int16)
    op_cols = []   # (col0, ncols16, n_idx, nt, nb) per op
    c0 = 0
    for g in range(N_OPS):
        nb = min(G_BLK, BLOCKS - g * G_BLK)
        n_idx = nb * T_B * 128
        op_cols.append((c0, n_idx // 16, n_idx, nb * T_B, nb))
        c0 += n_idx // 16
    for c in range(N_CORES):
        pos = 0
        for (col0, nc16, n_idx, nt, nb) in op_cols:
            a = sp[c, pos:pos + n_idx].reshape(n_idx // 16, 16).T
            gsrc[c, :, col0:col0 + nc16] = np.tile(
                a.astype(np.int16), (8, 1))
            pos += n_idx

    bs_pt = bs.reshape(N_CORES, T_total, 128).transpose(0, 2, 1).copy()
    dr_pt = dr.reshape(N_CORES, T_total, 128).transpose(0, 2, 1)
    dr_pt = dr_pt.astype(BF)
    n3 = (BLOCKS + 2) // 3
    drT = np.zeros((N_CORES, 128, n3 * T_B * 128), np.float32)
    drb = dr.reshape(N_CORES, BLOCKS, T_B * 128)
    for b in range(BLOCKS):
        drT[:, (b % 3) * 32, (b // 3) * T_B * 128:(b // 3 + 1) * T_B * 128] \
            = drb[:, b]
    drT = drT.astype(BF)

    xs = np.asarray(x, np.float32)
    x_sh = np.zeros((N_CORES, BLOCKS * 128, IN_FEAT), np.float32)
    x_sh[:, :NPC] = xs.reshape(N_CORES, NPC, IN_FEAT)
    x_sh = x_sh.astype(BF)

    return dict(T_B=T_B, T_total=T_total, SLOTS=SLOTS, op_cols=op_cols,
                gsrc=gsrc, bs_pt=bs_pt, dr_pt=dr_pt, drT=drT, x_sh=x_sh,
                inv=inv)


def _weights(ins, T_B):
    f32 = np.float32
    g = {k: np.asarray(v) for k, v in ins.items()}
    v = np.maximum(g["nn_w1"][0] + g["nn_b1"], 0.0)
    W = (v @ g["nn_w2"] + g["nn_b2"]).reshape(H_DIM, H_DIM)
    Wcat = np.concatenate([W, g["conv_root"]], 0).astype(BF)      # [64, 32]
    w0 = g["lin0_w"].astype(BF)                                   # [64, 32]
    b0 = np.tile(g["lin0_b"][None, :], (P, 1)).astype(f32)        # [128, 32]
    cb = np.tile(g["conv_bias"][None, :], (P, 1)).astype(f32)     # [128, 32]
    w1 = np.zeros((P, 24), f32)                # block-diag: 3 tiles/chunk
    for k in range(3):
        w1[k * 32:(k + 1) * 32, k * 8:(k + 1) * 8] = g["lin1_w"]
    w1 = w1.astype(BF)
    nt_max = G_BLK * T_B
    b1 = np.tile(np.tile(g["lin1_b"], nt_max)[None, :], (P, 1)).astype(f32)
    w2 = np.tile(np.tile(g["lin2_w"][:, 0], nt_max)[None, :], (P, 1)).astype(f32)
    b2 = np.full((P, 1), float(g["lin2_b"].reshape(-1)[0]), f32)
    iota_f = np.tile(np.arange(P, dtype=f32)[None, :], (P, 1)).astype(BF)
    iota_p = np.ascontiguousarray(
        np.tile(np.arange(P, dtype=f32)[:, None], (1, P))).astype(BF)
    iota_pw = np.ascontiguousarray(np.tile(
        np.arange(P, dtype=f32)[:, None], (1, G_BLK * T_B * 128))).astype(BF)
    on1 = np.ones((P, P), f32).astype(BF)
    return dict(Wcat=Wcat, w0=w0, b0=b0, cb=cb, w1=w1, b1=b1, w2=w2, b2=b2,
                iota_f=iota_f, iota_p=iota_p, iota_pw=iota_pw, on1=on1)


def _build(T_B, k_rep=1, phases=3, p1step=99, p2step=99):
    import concourse.bacc as bacc
    import concourse.mybir as mybir
    import concourse.tile as tile
    from concourse.masks import make_identity

    f32 = mybir.dt.float32
    bf16 = mybir.dt.bfloat16
    T_total = BLOCKS * T_B
    SLOTS = T_total * 128
    EPB = T_B * 128                 # edge slots per block
    nt_max = G_BLK * T_B

    nc = bacc.Bacc("TRN2", target_bir_lowering=False, debug=False,
                   num_devices=N_CORES, num_swdge_queues=4)
    dt = nc.dram_tensor
    x_d = dt("x_sh", [BLOCKS * 128, IN_FEAT], bf16, kind="ExternalInput")
    gsrc_d = dt("gsrc", [128, SLOTS // 16], mybir.dt.int16,
                kind="ExternalInput")
    bs_d = dt("bs_pt", [128, T_total], mybir.dt.uint8, kind="ExternalInput")
    dr_d = dt("dr_pt", [128, T_total], bf16, kind="ExternalInput")
    N3 = (BLOCKS + 2) // 3
    drT_d = dt("drT", [128, N3 * EPB], bf16, kind="ExternalInput")
    wc_d = dt("Wcat", [64, 32], bf16, kind="ExternalInput")
    w0_d = dt("w0", [64, 32], bf16, kind="ExternalInput")
    b0_d = dt("b0", [P, 32], f32, kind="ExternalInput")
    cb_d = dt("cb", [P, 32], f32, kind="ExternalInput")
    w1_d = dt("w1", [P, 24], bf16, kind="ExternalInput")
    b1_d = dt("b1", [P, nt_max * 8], f32, kind="ExternalInput")
    w2_d = dt("w2", [P, nt_max * 8], f32, kind="ExternalInput")
    iof_d = dt("iota_f", [P, P], bf16, kind="ExternalInput")
    iop_d = dt("iota_p", [P, P], bf16, kind="ExternalInput")
    iopw_d = dt("iota_pw", [P, nt_max * 128], bf16, kind="ExternalInput")
    on1_d = dt("on1", [P, P], bf16, kind="ExternalInput")
    b2_d = dt("b2", [P, 1], f32, kind="ExternalInput")

    h_full = dt("h_full", [TAB_ROWS * 128], bf16)
    o_full = dt("o_full", [TAB_ROWS * 128], bf16)
    sc_d = dt("scores", [P, T_total], f32, kind="ExternalOutput")

    bypass = mybir.AluOpType.bypass
    add = mybir.AluOpType.add
    mult = mybir.AluOpType.mult
    iseq = mybir.AluOpType.is_equal
    Relu = mybir.ActivationFunctionType.Relu
    Copy = mybir.ActivationFunctionType.Copy
    X = mybir.AxisListType.X

    sem_hr = nc.alloc_semaphore("sem_hr")   # h broadcast arrivals
    sem_or = nc.alloc_semaphore("sem_or")   # o broadcast arrivals
    sem_snd = nc.alloc_semaphore("sem_snd")
    sem_hcp = nc.alloc_semaphore("sem_hcp")
    sem_ocp = nc.alloc_semaphore("sem_ocp")

    # persistent SBUF (raw, visible across TileContexts)
    hs_tab = nc.alloc_sbuf_tensor([P, BLOCKS * 32], bf16)
    ob_tab = nc.alloc_sbuf_tensor([P, BLOCKS * 32], bf16)
    stg_h = nc.alloc_sbuf_tensor([P, 7 * BLOCKS * 32], bf16)
    stg_o = nc.alloc_sbuf_tensor([P, 7 * BLOCKS * 32], bf16)
    hxT = nc.alloc_sbuf_tensor([64, BLOCKS * 128], bf16)
    bs_sb = nc.alloc_sbuf_tensor([P, T_total], mybir.dt.uint8)
    dr_sb = nc.alloc_sbuf_tensor([P, T_total], bf16)
    ident = nc.alloc_sbuf_tensor([P, P], bf16)
    wc_sb = nc.alloc_sbuf_tensor([64, 32], bf16)
    w0_sb = nc.alloc_sbuf_tensor([64, 32], bf16)
    b0_sb = nc.alloc_sbuf_tensor([P, 32], f32)
    cb_sb = nc.alloc_sbuf_tensor([P, 32], f32)
    w1_sb = nc.alloc_sbuf_tensor([P, 24], bf16)
    b1_sb = nc.alloc_sbuf_tensor([P, nt_max * 8], f32)
    w2_sb = nc.alloc_sbuf_tensor([P, nt_max * 8], f32)
    iof_sb = nc.alloc_sbuf_tensor([P, P], bf16)
    iop_sb = nc.alloc_sbuf_tensor([P, P], bf16)
    iopw_sb = nc.alloc_sbuf_tensor([P, nt_max * 128], bf16)
    drT_sb = nc.alloc_sbuf_tensor([P, ((BLOCKS + 2) // 3) * EPB], bf16)
    on1_sb = nc.alloc_sbuf_tensor([P, P], bf16)
    b2_sb = nc.alloc_sbuf_tensor([P, 1], f32)

    def bcast(tab_ap, stg, chunk_cols, ksem):
        """Send tab cols to all 7 peers' stage slots (XOR-slot layout)."""
        c0, c1 = chunk_cols
        for e in range(1, N_CORES):
            pe = e ^ 2 if e & 4 else e      # logical->physical tpb distance
            rdests = [None] * 8
            rdests[pe] = (0, pe)
            nc.gpsimd.remote_dma_broadcast(
                out_ap=stg[:][:, (e - 1) * BLOCKS * 32 + c0:
                              (e - 1) * BLOCKS * 32 + c1],
                in_ap=tab_ap[:][:, c0:c1],
                remote_sem=ksem,
                local_sem=sem_snd,
                rdests=rdests,
            )
        nc.gpsimd.trigger_dma(count=7)

    def own_slot_dma(dram_flat, tab, b0, nb, ksem):
        """Write own blocks b0..b0+nb as [n0 n0 n1 n1] pair rows, slot 0."""
        src = tab[:][:, b0 * 32:(b0 + nb) * 32].rearrange(
            "p (b f) -> p b f", f=32)
        for rep in range(2):
            out = dram_flat[b0 * 8192:(b0 + nb) * 8192].rearrange(
                "(b p f) -> p b f", p=128, f=64)[:, :, rep * 32:rep * 32 + 32]
            nc.sync.dma_start(out=out, in_=src).then_inc(ksem, 16)

    def slot_copy_dma(dram_flat, stg, ksem):
        """Copy the 7 received stage slots into DRAM table slots 1..7."""
        for e in range(1, N_CORES):
            src = stg[:][:, (e - 1) * BLOCKS * 32:e * BLOCKS * 32].rearrange(
                "p (b f) -> p b f", f=32)
            base = e * PAIRS_SLOT * 128
            for rep in range(2):
                out = h_o_view(dram_flat, base)[
                    :, :, rep * 32:rep * 32 + 32]
                nc.sync.dma_start(out=out, in_=src).then_inc(ksem, 16)

    def h_o_view(dram_flat, base):
        return dram_flat[base:base + BLOCKS * 8192].rearrange(
            "(b p f) -> p b f", p=128, f=64)

    with tile.TileContext(nc) as tc0:
        with tc0.tile_pool(name="ld", bufs=1):
            make_identity(nc, ident[:])
            for sb, d in [(bs_sb, bs_d), (dr_sb, dr_d), (wc_sb, wc_d), (w0_sb, w0_d), (b0_sb, b0_d),
                          (cb_sb, cb_d), (w1_sb, w1_d), (b1_sb, b1_d),
                          (w2_sb, w2_d), (iof_sb, iof_d), (iop_sb, iop_d),
                          (iopw_sb, iopw_d), (drT_sb, drT_d),
                          (on1_sb, on1_d), (b2_sb, b2_d)]:
                nc.sync.dma_start(out=sb[:], in_=d[:])

    for rep in range(k_rep):
        # ---------------- phase 0: h = relu(x @ w0 + b0) ---------------
        with tile.TileContext(nc) as tc:
            with (
                tc.tile_pool(name="p0", bufs=3) as p0,
                tc.tile_pool(name="p0p", bufs=2, space="PSUM") as p0p,
            ):
                for b in range(BLOCKS):
                    xt = p0.tile([P, IN_FEAT], bf16, tag="xt")
                    nc.sync.dma_start(
                        out=xt[:], in_=x_d[b * 128:(b + 1) * 128, :])
                    ps_xT = p0p.tile([IN_FEAT, P], bf16, tag="xT")
                    nc.tensor.transpose(out=ps_xT[:], in_=xt[:],
                                        identity=ident[:])
                    xT = p0.tile([IN_FEAT, P], bf16, tag="xTs")
                    nc.scalar.activation(out=xT[:], in_=ps_xT[:], func=Copy)
                    ps_h = p0p.tile([P, 32], f32, tag="h")
                    nc.tensor.matmul(out=ps_h[:], lhsT=xT[:],
                                     rhs=w0_sb[:], start=True, stop=True)
                    hb = p0.tile([P, 32], bf16, tag="hb")
                    nc.vector.tensor_tensor(out=hb[:], in0=ps_h[:],
                                            in1=b0_sb[:], op=add)
                    nc.scalar.activation(
                        out=hs_tab[:][:, b * 32:(b + 1) * 32],
                        in_=hb[:], func=Relu)
                    ps_hT = p0p.tile([32, P], bf16, tag="hT")
                    nc.tensor.transpose(
                        out=ps_hT[:],
                        in_=hs_tab[:][:, b * 32:(b + 1) * 32],
                        identity=ident[:])
                    nc.scalar.activation(
                        out=hxT[:][32:64, b * 128:(b + 1) * 128],
                        in_=ps_hT[:], func=Copy)
                    if b == CH_BLKS - 1:
                        bcast(hs_tab, stg_h, (0, CH_BLKS * 32), sem_hr)
                    if b == BLOCKS - 1:
                        bcast(hs_tab, stg_h,
                              (CH_BLKS * 32, BLOCKS * 32), sem_hr)
        # raw: wait h arrivals, copy stage -> table slots
        own_slot_dma(h_full, hs_tab, 0, BLOCKS, sem_hcp)
        nc.sync.wait_ge(sem_hr, (rep + 1) * 28)
        slot_copy_dma(h_full, stg_h, sem_hcp)
        nc.sync.wait_ge(sem_hcp, (rep + 1) * 16 * 16)

        if phases < 1:
            continue
        # ------- phase 1: gather h[src], aggregate, out table ----------
        with tile.TileContext(nc) as tc:
            with (
                tc.tile_pool(name="p1", bufs=3) as p1,
                tc.tile_pool(name="p1b", bufs=2) as p1b,
                tc.tile_pool(name="p1p", bufs=2, space="PSUM") as p1p,
                tc.tile_pool(name="p1q", bufs=2, space="PSUM") as p1q,
            ):
                for g in range(N_OPS):
                    col0 = g * G_BLK * T_B * 8
                    nb = min(G_BLK, BLOCKS - g * G_BLK)
                    nt = nb * T_B
                    n_idx = nt * 128
                    t0g = g * G_BLK * T_B
                    gi = p1.tile([128, nt_max * 8], mybir.dt.int16, tag="gi")
                    nc.sync.dma_start(
                        out=gi[:, :n_idx // 16],
                        in_=gsrc_d[:, col0:col0 + n_idx // 16])
                    gd = p1.tile([P, nt_max, 128], bf16, tag="gd")
                    for ci, ts in enumerate(range(0, nt, 7)):
                        tn = min(7, nt - ts)
                        nc.gpsimd.dma_gather(
                            gd[:, ts:ts + tn, :], h_full[:].rearrange(
                                "(r c) -> r c", c=128),
                            gi[:, ts * 8:(ts + tn) * 8],
                            tn * 128, tn * 128, 128, queue_num=ci % 4)
                    if p1step < 1:
                        continue
                    A = p1.tile([P, nt_max, 32], bf16, tag="A")
                    nc.vector.tensor_copy(out=A[:, :nt, :],
                                          in_=gd[:, :nt, 0:32])
                    mk = bs_sb[:][:, t0g:t0g + nt].unsqueeze(2).to_broadcast(
                        [P, nt, 32])
                    nc.vector.copy_predicated(out=A[:, :nt, :], mask=mk,
                                              data=gd[:, :nt, 64:96])
                    if p1step < 2:
                        continue
                    oh = p1b.tile([P, nt_max * 128], bf16, tag="oh")
                    nc.vector.tensor_tensor(
                        out=oh[:, :nt * 128].rearrange(
                            "p (t f) -> p t f", f=128),
                        in0=dr_sb[:][:, t0g:t0g + nt].unsqueeze(
                            2).to_broadcast([P, nt, 128]),
                        in1=iof_sb[:].unsqueeze(1).to_broadcast(
                            [P, nt, 128]),
                        op=iseq)
                    if p1step < 3:
                        continue
                    ps_g0T = None
                    for t in range(nt):
                        j = t % T_B
                        b = g * G_BLK + t // T_B
                        if j == 0:
                            ps_g0T = p1p.tile([32, P], f32, tag="g0T")
                        nc.tensor.matmul(
                            out=ps_g0T[:], lhsT=A[:, t, :],
                            rhs=oh[:, t * 128:(t + 1) * 128],
                            start=(j == 0), stop=(j == T_B - 1))
                        if j == T_B - 1:
                            nc.scalar.activation(
                                out=hxT[:][0:32, b * 128:(b + 1) * 128],
                                in_=ps_g0T[:], func=Copy)
                            ps_o = p1q.tile([P, 32], f32, tag="o")
                            nc.tensor.matmul(
                                out=ps_o[:],
                                lhsT=hxT[:][:, b * 128:(b + 1) * 128],
                                rhs=wc_sb[:], start=True, stop=True)
                            nc.vector.tensor_tensor(
                                out=ob_tab[:][:, b * 32:(b + 1) * 32],
                                in0=ps_o[:], in1=cb_sb[:], op=add)
                    if p1step < 4:
                        continue
                    if g == CH_BLKS // G_BLK - 1:
                        bcast(ob_tab, stg_o, (0, CH_BLKS * 32), sem_or)
                    if g == N_OPS - 1:
                        bcast(ob_tab, stg_o,
                              (CH_BLKS * 32, BLOCKS * 32), sem_or)
        if p1step >= 4:
            own_slot_dma(o_full, ob_tab, 0, BLOCKS, sem_ocp)
            nc.sync.wait_ge(sem_or, (rep + 1) * 28)
            slot_copy_dma(o_full, stg_o, sem_ocp)
            nc.sync.wait_ge(sem_ocp, (rep + 1) * 16 * 16)

        if phases < 2:
            continue
        # ---------------- phase 2: edge scores -------------------------
        with tile.TileContext(nc) as tc:
            with (
                tc.tile_pool(name="p2", bufs=3) as p2,
                tc.tile_pool(name="p2b", bufs=2) as p2b,
                tc.tile_pool(name="p2p", bufs=2, space="PSUM") as p2p,
                tc.tile_pool(name="p2q", bufs=2, space="PSUM") as p2q,
            ):
                for g in range(N_OPS):
                    col0 = g * G_BLK * T_B * 8
                    nb = min(G_BLK, BLOCKS - g * G_BLK)
                    nt = nb * T_B
                    n_idx = nt * 128
                    t0g = g * G_BLK * T_B
                    gi = p2.tile([128, nt_max * 8], mybir.dt.int16, tag="gi2")
                    nc.sync.dma_start(
                        out=gi[:, :n_idx // 16],
                        in_=gsrc_d[:, col0:col0 + n_idx // 16])
                    gd = p2.tile([P, nt_max, 128], bf16, tag="gd2")
                    for ci, ts in enumerate(range(0, nt, 7)):
                        tn = min(7, nt - ts)
                        nc.gpsimd.dma_gather(
                            gd[:, ts:ts + tn, :], o_full[:].rearrange(
                                "(r c) -> r c", c=128),
                            gi[:, ts * 8:(ts + tn) * 8],
                            tn * 128, tn * 128, 128, queue_num=ci % 4)
                    A2 = p2.tile([P, nt_max, 32], bf16, tag="A2")
                    nc.vector.tensor_copy(out=A2[:, :nt, :],
                                          in_=gd[:, :nt, 0:32])
                    mk = bs_sb[:][:, t0g:t0g + nt].unsqueeze(2).to_broadcast(
                        [P, nt, 32])
                    nc.vector.copy_predicated(out=A2[:, :nt, :], mask=mk,
                                              data=gd[:, :nt, 64:96])
                    if p2step < 2:
                        continue
                    # bc: per block, broadcast dst_rel row to 128 partitions
                    bc = p2b.tile([P, nt_max * 128], bf16, tag="bc")
                    for bi in range(nb):
                        b = g * G_BLK + bi
                        for k in range(3):
                            cw = EPB // 3
                            ps_bc = p2q.tile([P, 512], f32, tag="bc")
                            q = (b % 3) * 32
                            c0d = (b // 3) * EPB
                            nc.tensor.matmul(
                                out=ps_bc[:, :cw],
                                lhsT=on1_sb[q:q + 1, :],
                                rhs=drT_sb[q:q + 1,
                                           c0d + k * cw:c0d + (k + 1) * cw],
                                start=True, stop=True)
                            nc.scalar.activation(
                                out=bc[:, bi * EPB + k * cw:
                                       bi * EPB + (k + 1) * cw],
                                in_=ps_bc[:, :cw], func=Copy)
                    if p2step < 3:
                        continue
                    ohT = p2b.tile([P, nt_max * 128], bf16, tag="ohT")
                    nc.vector.tensor_tensor(
                        out=ohT[:, :nt * 128], in0=bc[:, :nt * 128],
                        in1=iopw_sb[:][:, :nt * 128],
                        op=iseq)
                    if p2step < 4:
                        continue
                    ps_m = p2p.tile([P, nt_max * 8], f32, tag="m")
                    for bi in range(nb):
                        b = g * G_BLK + bi
                        ps_B = p2p.tile([P, T_B, 33], f32, tag="B")
                        for j in range(T_B):
                            nc.tensor.matmul(
                                out=ps_B[:, j, 0:32],
                                lhsT=ohT[:, (bi * T_B + j) * 128:
                                         (bi * T_B + j + 1) * 128],
                                rhs=ob_tab[:][:, b * 32:(b + 1) * 32],
                                start=True, stop=True)
                        if p2step < 5:
                            continue
                        z = p2.tile([P, T_B * 32], bf16, tag="z")
                        nc.vector.tensor_tensor(
                            out=z[:].rearrange("p (t f) -> p t f", f=32),
                            in0=A2[:, bi * T_B:(bi + 1) * T_B, :],
                            in1=ps_B[:, :, 0:32], op=mult)
                        if p2step < 6:
                            continue
                        for k in range(3):
                            ps_zT = p2q.tile([96, P], bf16, tag="zT")
                            nc.tensor.transpose(
                                out=ps_zT[:], in_=z[:, k * 96:(k + 1) * 96],
                                identity=ident[:])
                            zT = p2.tile([96, P], bf16, tag="zTs")
                            nc.scalar.activation(out=zT[:], in_=ps_zT[:],
                                                 func=Copy)
                            if p2step < 7:
                                continue
                            tl0 = bi * T_B + k * 3
                            nc.tensor.matmul(
                                out=ps_m[:, tl0 * 8:(tl0 + 3) * 8],
                                lhsT=zT[:], rhs=w1_sb[0:96, :],
                                start=True, stop=True)
                    if p2step < 8:
                        continue
                    s1 = p2.tile([P, nt_max * 8], f32, tag="s1")
                    nc.vector.tensor_tensor(out=s1[:, :nt * 8],
                                            in0=ps_m[:, :nt * 8],
                                            in1=b1_sb[:][:, :nt * 8], op=add)
                    s1r = p2.tile([P, nt_max * 8], f32, tag="s1r")
                    nc.scalar.activation(out=s1r[:, :nt * 8],
                                         in_=s1[:, :nt * 8], func=Relu)
                    nc.vector.tensor_tensor(out=s1r[:, :nt * 8],
                                            in0=s1r[:, :nt * 8],
                                            in1=w2_sb[:][:, :nt * 8],
                                            op=mult)
                    sc = p2.tile([P, nt_max], f32, tag="sc")
                    nc.vector.reduce_sum(
                        out=sc[:, :nt],
                        in_=s1r[:, :nt * 8].rearrange(
                            "p (t e) -> p t e", e=8),
                        axis=X)
                    sc2 = p2.tile([P, nt_max], f32, tag="sc2")
                    nc.vector.tensor_tensor(
                        out=sc2[:, :nt], in0=sc[:, :nt],
                        in1=b2_sb[:][:, 0:1].to_broadcast([P, nt]), op=add)
                    nc.sync.dma_start(out=sc_d[:, t0g:t0g + nt],
                                      in_=sc2[:, :nt])
    nc.compile()
    return nc


def _in_maps(prep, wts):
    maps = []
    for c in range(N_CORES):
        maps.append({
            "x_sh": prep["x_sh"][c],
            "gsrc": prep["gsrc"][c],
            "bs_pt": prep["bs_pt"][c],
            "dr_pt": prep["dr_pt"][c],
            "drT": prep["drT"][c],
            "Wcat": wts["Wcat"], "w0": wts["w0"], "b0": wts["b0"],
            "cb": wts["cb"], "w1": wts["w1"], "b1": wts["b1"],
            "w2": wts["w2"], "iota_f": wts["iota_f"],
            "iota_p": wts["iota_p"], "iota_pw": wts["iota_pw"],
            "on1": wts["on1"],
            "b2": wts["b2"],
        })
    return maps


def _assemble(results, prep):
    scores = np.empty(N_EDGES, np.float32)
    for c in range(N_CORES):
        flat = np.asarray(results[c]["scores"], np.float32).T.reshape(-1)
        inv = prep["inv"][c]
        m = inv >= 0
        scores[inv[m]] = flat[m]
    return scores


def kernel(**inputs):
    from concourse.bass_utils import run_bass_kernel_spmd
    prep = _prep(inputs["x"], inputs["edge_index"])
    wts = _weights(inputs, prep["T_B"])
    nc = _build(prep["T_B"], k_rep=1)
    res = run_bass_kernel_spmd(nc, _in_maps(prep, wts),
                               list(range(N_CORES)))
    return _assemble(res.results, prep)

